# revision 18
# baseline (speedup 1.0000x reference)
"""2-layer GCN (GCNConv x2 + log_softmax) on 8 Trainium2 NeuronCores.

v9: the per-edge gather moves from GPSIMD ap_gather (28ns/idx serial on the
Q7s) to SWDGE dma_gather in transpose mode: 256B HBM table rows sprayed
across 128 partitions, 512-idx calls round-robin over 4 SWDGE queues (each
queue = its own Q7 core pair, so all 8 Q7s generate descriptors in
parallel; measured issue rate ~0.3-0.6ns/idx vs ap_gather's 3.5ns/edge
effective).  Table rows interleave all 8 source-group feature sets
(partition 16g+f = feature f of group g), built per layer by PE-transposing
the AllGathered [128, S] bf16 feature block into DRAM [S, 128].  Each group
stream's gather output carries its 16-partition block; a block-identity PE
matmul consolidates the 8 streams' blocks into one [128, CH] psum chunk,
then the DVE class-run reduce produces classout exactly as before.  Edge
streams are packed into whole-segment 512-idx chunks (classes descending,
~1% pad) shared by all 64 (core, group) pairs.  The prior session's
"dma_gather deadlocks" finding was the SWDGE ring-capacity limit: one call
must stay under ~896 idx; chunked calls with cross-call reclaim are fine.
Layer-1 un-permute stays on ap_gather (12544 idx); layer 2 still ships
class-ordered partials + g2 to the host (v4 scheme).
"""
import numpy as np

NC = 8            # cores
NG = 8            # source groups (= shards)
S = 12544         # nodes per shard
V = NC * S        # padded node count
N_REAL = 100000
F = 16            # hidden dim
C8 = 8            # padded class count
X = 512           # input dim
TILE = 512        # post-phase column tile
CH = 384          # slots per chunk per group (call = 2*CH = 768 idx)
ELEM = 128        # bf16 elements per table row = 256B


# --------------------------------------------------------------- tile patch
def _install_tile_patch():
    """The Tile tail drain accumulates more sem waits than this compiler
    allows on one CTRL instruction; spread them over SP nops (1 wait each)."""
    import concourse.tile as tile
    import concourse.mybir as mybir
    from concourse.vector_clock import ScopedClock
    if getattr(tile.TileContext, "_drain_patch", False):
        return
    _MAX_WAITS = 1

    def _patched(self, tick_clock, wait_clock):
        nc = self.nc
        nops = [nc.sync.nop(nofuse=True) for _ in range(40)]
        drain_inst = nc.sync.drain()
        wait_clock.add_sem_waits(
            drain_inst.ins, ScopedClock({None: tick_clock.global_clock})
        )
        si = drain_inst.ins.sync_info
        if si is not None and si.on_wait and len(si.on_wait) > _MAX_WAITS:
            waits = list(si.on_wait)
            si.on_wait.clear()
            chunks = [waits[i:i + _MAX_WAITS] for i in range(0, len(waits), _MAX_WAITS)]
            si.on_wait.extend(chunks[-1])
            rest = chunks[:-1]
            assert len(rest) <= len(nops), f"too many wait chunks: {len(rest)}"
            for nop, chunk in zip(nops, rest):
                nsi = nop.ins.sync_info
                if nsi is None:
                    nop.ins.sync_info = mybir.SyncInfo(on_wait=list(chunk), on_update=[])
                else:
                    nsi.on_wait.extend(chunk)
        nc.all_engine_barrier()
        assert self.sems is not None
        popped = nc._tile_sem_poison_stack.pop()
        assert popped is self._sem_poison
        nc.clear_and_free_semaphores(list(self.sems.allocated().values()))
        nc.all_engine_barrier()

    tile.TileContext._drain_and_barrier = _patched
    tile.TileContext._drain_patch = True


# ---------------------------------------------------------------- host prep
def preprocess(edge_index):
    row = np.asarray(edge_index[0], dtype=np.int64)
    col = np.asarray(edge_index[1], dtype=np.int64)
    deg_full = (np.bincount(col, minlength=V) + 1).astype(np.int32)

    core_of = (col // S).astype(np.int32)
    per_core = []
    cnt_all = np.zeros((NC, NG, S), dtype=np.int64)
    for k in range(NC):
        m = core_of == k
        r, c = row[m], col[m]
        g = (r // S).astype(np.int64)
        sloc = (r - g * S).astype(np.int64)
        dloc = (c - k * S).astype(np.int64)
        cnt = np.bincount(g * S + dloc, minlength=NG * S).reshape(NG, S)
        cnt_all[k] = cnt
        per_core.append((g, sloc, dloc, cnt))

    cmax = int(cnt_all.max())
    ncls = np.zeros((NC, NG, cmax + 1), dtype=np.int64)
    for k in range(NC):
        for g in range(NG):
            ncls[k, g] = np.bincount(cnt_all[k, g], minlength=cmax + 1)
    n_glob = ncls.max(axis=(0, 1))
    classes = [c for c in range(1, cmax + 1) if n_glob[c] > 0]

    # --- shared chunk plan: whole segments packed into CH-idx chunks,
    # classes descending, position space assigned in packing order ---
    plan_chunks = []      # per chunk: list of (loff, c, nseg, pos)
    slot_map = {}         # c -> [(chunk_id, loff_of_slot)]
    pos_base = {}
    cur_runs, used = [], 0
    pos_next = 1          # position 0 reserved as zero
    for c in sorted(classes, reverse=True):
        n = int(n_glob[c])
        pos_base[c] = pos_next
        slots_c = []
        placed = 0
        while placed < n:
            space = CH - used
            kfit = min(n - placed, space // c)
            if kfit == 0:
                plan_chunks.append(cur_runs)
                cur_runs, used = [], 0
                continue
            cur_runs.append((used, c, kfit, pos_next))
            for j in range(kfit):
                slots_c.append((len(plan_chunks), used + j * c))
            used += kfit * c
            pos_next += kfit
            placed += kfit
        slot_map[c] = slots_c
    if cur_runs:
        plan_chunks.append(cur_runs)
    nchunks = len(plan_chunks)
    L = nchunks * CH
    T = pos_next
    assert T < 32768 and L % 16 == 0

    inputs = []
    _pos_arrays = []
    for k in range(NC):
        g, sloc, dloc, cnt = per_core[k]
        streams = np.zeros((NG, L), dtype=np.int16)
        pos_arr = np.zeros((NG, S), dtype=np.int16)
        for gg in range(NG):
            m = g == gg
            sl, dl = sloc[m], dloc[m]
            cc = cnt[gg, dl]
            order = np.lexsort((sl, dl, cc))
            sl, cc = sl[order], cc[order]
            bnd = np.searchsorted(cc, np.arange(1, cmax + 2))
            for c in classes:
                lo, hi = bnd[c - 1], bnd[c]
                nreal = (hi - lo) // c
                slots = slot_map[c]
                for j in range(nreal):
                    cid, loff = slots[j]
                    streams[gg, cid * CH + loff: cid * CH + loff + c] = \
                        sl[lo + j * c: lo + (j + 1) * c]
            # output positions per dest (rank within class by dest order)
            arr = cnt[gg]
            sorted_d = np.argsort(arr, kind="stable")
            arr_s = arr[sorted_d]
            first = np.searchsorted(arr_s, arr_s)
            rank = np.arange(S) - first
            ps = np.zeros(S, dtype=np.int64)
            starts = np.array([pos_base[int(c)] if c > 0 else 0 for c in arr_s])
            ps[sorted_d] = np.where(arr_s > 0, starts + rank, 0)
            pos_arr[gg] = ps.astype(np.int16)

        # dma_gather idx tensor: queue q reads partitions [32q, 32q+32);
        # one 768-idx call per queue per chunk = [group q | group 4+q]
        W2C = 2 * CH // 16
        idxC = np.zeros((128, nchunks * W2C), np.int16)
        for q in range(4):
            for ci in range(nchunks):
                cat = np.concatenate([streams[q, ci * CH:(ci + 1) * CH],
                                      streams[4 + q, ci * CH:(ci + 1) * CH]])
                wr = cat.reshape(2 * CH // 16, 16).T     # [16, 48]
                idxC[32 * q:32 * q + 16, ci * W2C:(ci + 1) * W2C] = wr
                idxC[32 * q + 16:32 * q + 32, ci * W2C:(ci + 1) * W2C] = wr

        def pack16(mat, width):
            out = np.zeros((128, width // 16), np.int16)
            for gg in range(NG):
                out[16 * gg:16 * gg + 16, :] = mat[gg].reshape(width // 16, 16).T
            return out

        _pos_arrays.append(pos_arr.astype(np.int32))
        inputs.append({
            "idxC": idxC,
            "unperm": pack16(pos_arr, S),
            "dinv_in": (1.0 / np.sqrt(deg_full[k * S:(k + 1) * S].astype(np.float64))
                        ).astype(np.float32).reshape(1, S),
        })
    meta = dict(L=L, T=T, nchunks=nchunks, plan=plan_chunks,
                pos=[p for p in _pos_arrays])
    return inputs, meta


# ---------------------------------------------------------------- kernel
def build_kernel(meta, reps=1, debug_co1=False, only_batch_a=False):
    import concourse.bass as bass
    import concourse.mybir as mybir
    from concourse import bacc
    from concourse.tile import TileContext
    _install_tile_patch()
    AF = mybir.ActivationFunctionType
    DT = mybir.dt
    L, T = meta["L"], meta["T"]
    nchunks, plan = meta["nchunks"], meta["plan"]

    nc = bacc.Bacc(None, target_bir_lowering=False, num_devices=NC,
                   num_swdge_queues=4)
    W2C = 2 * CH // 16
    f32 = DT.float32
    bf16 = DT.bfloat16
    i16 = DT.int16

    xt_d = nc.dram_tensor("xt", [X, S], bf16, kind="ExternalInput")
    dinv_in_d = nc.dram_tensor("dinv_in", [1, S], f32, kind="ExternalInput")
    idxC_d = nc.dram_tensor("idxC", [128, 2 * L // 16], i16,
                            kind="ExternalInput")
    unp_d = nc.dram_tensor("unperm", [128, S // 16], i16, kind="ExternalInput")
    w1_d = nc.dram_tensor("W1", [X, F], bf16, kind="ExternalInput")
    b1_d = nc.dram_tensor("b1", [F, 1], f32, kind="ExternalInput")
    w2_d = nc.dram_tensor("W2", [F, C8], f32, kind="ExternalInput")
    pcomb_d = nc.dram_tensor("pcomb", [128, F], f32, kind="ExternalInput")
    identb_d = nc.dram_tensor("identb", [128, 128], bf16, kind="ExternalInput")
    selm_d = nc.dram_tensor("selm", [128, 1024], bf16, kind="ExternalInput")
    co2_d = nc.dram_tensor("co2_t", [128, T], f32, kind="ExternalOutput")
    co1_d = (nc.dram_tensor("co1_t", [128, T], f32, kind="ExternalOutput")
             if debug_co1 else None)
    g2_d = nc.dram_tensor("g2_t", [C8, S], f32, kind="ExternalOutput")

    sems = [nc.alloc_semaphore(f"dgsem{q}") for q in range(4)]
    nq = [0, 0, 0, 0]

    def widths():
        off = 0
        while off < S:
            w = min(TILE, S - off)
            yield off, w
            off += w

    with TileContext(nc) as tc:
        with tc.tile_pool(name="dram", bufs=1, space="DRAM") as dram, \
             tc.tile_pool(name="const", bufs=1) as constp:
            tabs = [dram.tile([S, ELEM], bf16, name=f"tab{l}") for l in (1, 2)]

            idxC_t = constp.tile([128, 2 * L // 16], i16)
            nc.sync.dma_start(out=idxC_t[:], in_=idxC_d[:])
            unp_t = constp.tile([128, S // 16], i16)
            nc.sync.dma_start(out=unp_t[:], in_=unp_d[:])
            w1_t = constp.tile([128, 4 * F], bf16)
            for kc in range(4):
                nc.sync.dma_start(out=w1_t[:, kc * F:(kc + 1) * F],
                                  in_=w1_d[kc * 128:(kc + 1) * 128, :])
            w2_t = constp.tile([F, C8], f32)
            nc.sync.dma_start(out=w2_t[:], in_=w2_d[:])
            b1_t = constp.tile([F, 1], f32)
            nc.sync.dma_start(out=b1_t[:], in_=b1_d[:])
            pcomb_t = constp.tile([128, F], f32)
            nc.sync.dma_start(out=pcomb_t[:], in_=pcomb_d[:])
            identb_t = constp.tile([128, 128], bf16)
            nc.sync.dma_start(out=identb_t[:], in_=identb_d[:])
            selm_t = constp.tile([128, 1024], bf16)
            nc.sync.dma_start(out=selm_t[:], in_=selm_d[:])

            # warm-up: the first dma_gather(s) of an execution produce
            # corrupted rx sprays (startup race around library load / ring
            # init). Fire one sacrificial call per queue into scratch and
            # wait for full completion before any real gather.
            warm = constp.tile([128, 128], bf16)
            for q in range(4):
                nc.gpsimd.dma_gather(
                    warm[:].rearrange("p (o n) -> p o n", o=1),
                    tabs[0][:], idxC_t[:, 0:8], 128, 128, ELEM,
                    transpose=True, queue_num=q,
                ).then_inc(sems[q], 16)
                nq[q] += 1
            for q in range(4):
                nc.gpsimd.wait_ge(sems[q], 16 * nq[q])

            for _rep in range(reps):
                cc_in1 = dram.tile([F, S], bf16, name=f"cc_in1_r{_rep}")
                # 16 rows: 8 real classes + 8 zero rows so the AllGathered
                # [128, S] block has no garbage partitions (a NaN anywhere
                # would poison the 0/1 consolidation matmul: 0*NaN = NaN)
                cc_in2 = dram.tile([F, S], bf16, name=f"cc_in2_r{_rep}")
                cc_out1 = dram.tile([128, S], bf16, addr_space="Shared",
                                    name=f"cc_out1_r{_rep}")
                cc_out2 = dram.tile([128, S], bf16, addr_space="Shared",
                                    name=f"cc_out2_r{_rep}")

                # ---- phase 1: g1 = (dinv*x) @ W1  (bf16 out) ----
                # dma_gather transpose sprays corrupt under concurrent HWDGE
                # traffic: fence all bulk DMA behind the gather sems
                if _rep > 0:
                    for q in range(4):
                        nc.sync.wait_ge(sems[q], 16 * nq[q])
                with tc.tile_pool(name=f"mmx{_rep}", bufs=4) as mmx, \
                     tc.tile_pool(name=f"mmo{_rep}", bufs=3) as mmo, \
                     tc.tile_pool(name=f"ps1_{_rep}", bufs=3, space="PSUM") as ps1:
                    for off, w in widths():
                        psum = ps1.tile([F, TILE], f32, tag="ps")
                        for kc in range(4):
                            xt_t = mmx.tile([128, TILE], bf16, tag="xt")
                            nc.sync.dma_start(
                                out=xt_t[:, :w],
                                in_=xt_d[kc * 128:(kc + 1) * 128, off:off + w])
                            nc.tensor.matmul(
                                out=psum[:, :w],
                                lhsT=w1_t[:, kc * F:(kc + 1) * F],
                                rhs=xt_t[:, :w],
                                start=(kc == 0), stop=(kc == 3))
                        g1 = mmo.tile([F, TILE], bf16, tag="g1")
                        nc.vector.tensor_copy(out=g1[:, :w], in_=psum[:, :w])
                        nc.sync.dma_start(out=cc_in1[:, off:off + w],
                                          in_=g1[:, :w])

                for layer in (1, 2):
                    cc_in = cc_in1 if layer == 1 else cc_in2
                    cc_out = cc_out1 if layer == 1 else cc_out2
                    tab = tabs[layer - 1]
                    for q in range(4):
                        nc.gpsimd.wait_ge(sems[q], 16 * nq[q])
                    nc.gpsimd.collective_compute(
                        "AllGather", mybir.AluOpType.bypass,
                        replica_groups=[list(range(NC))],
                        ins=[cc_in[:]], outs=[cc_out[:]])

                    # ---- build interleaved table [S, 128] bf16 in DRAM ----
                    with tc.tile_pool(name=f"tb{layer}_{_rep}", bufs=1) as tbp, \
                         tc.tile_pool(name=f"tt{layer}_{_rep}", bufs=3) as ttp, \
                         tc.tile_pool(name=f"tp{layer}_{_rep}", bufs=3,
                                      space="PSUM") as tpp:
                        ts = tbp.tile([128, S], bf16)
                        nc.sync.dma_start(out=ts[:], in_=cc_out[:])
                        # prior-rep gathers must have drained before overwrite
                        if _rep > 0:
                            for q in range(4):
                                nc.sync.wait_ge(sems[q], 16 * nq[q])
                        for t in range(S // 128):
                            tps = tpp.tile([128, 128], bf16, tag="tp")
                            nc.tensor.transpose(
                                out=tps[:], in_=ts[:, 128 * t:128 * t + 128],
                                identity=identb_t[:])
                            tsb = ttp.tile([128, 128], bf16, tag="tsb")
                            nc.vector.tensor_copy(out=tsb[:], in_=tps[:])
                            nc.sync.dma_start(
                                out=tab[128 * t:128 * t + 128, :], in_=tsb[:])

                    # ---- gather + consolidate + class-reduce ----
                    with tc.tile_pool(name=f"cls{layer}_{_rep}", bufs=1) as clsp, \
                         tc.tile_pool(name=f"gt{layer}_{_rep}", bufs=3) as gtp, \
                         tc.tile_pool(name=f"gp{layer}_{_rep}", bufs=2,
                                      space="PSUM") as gpp:
                        classout = clsp.tile([128, T], f32)
                        nc.vector.memset(classout[:, 0:1], 0.0)
                        for ci in range(nchunks):
                            gts = []
                            for q in range(4):
                                gt = gtp.tile([128, 2 * CH], bf16, tag=f"g{q}",
                                              name=f"gt{q}")
                                nc.gpsimd.dma_gather(
                                    gt[:].rearrange("p (o n) -> p o n", o=1),
                                    tab[:],
                                    idxC_t[:, ci * W2C:(ci + 1) * W2C],
                                    2 * CH, 2 * CH, ELEM,
                                    transpose=True, queue_num=q,
                                ).then_inc(sems[q], 16)
                                nq[q] += 1
                                gts.append(gt)
                            for q in range(4):
                                nc.tensor.wait_ge(sems[q], 16 * nq[q])
                            cps = gpp.tile([128, CH], f32, tag="cps")
                            # consolidate the 8 streams' 16-partition blocks:
                            # one full-width accumulation group, 8 matmuls
                            # with block-masked identity selectors
                            for g in range(NG):
                                rhs = (gts[g][:, 0:CH] if g < 4
                                       else gts[g - 4][:, CH:2 * CH])
                                nc.tensor.matmul(
                                    out=cps[:],
                                    lhsT=selm_t[:, 128 * g:128 * g + 128],
                                    rhs=rhs,
                                    start=(g == 0), stop=(g == NG - 1))
                            for (loff, c, nseg, pos) in plan[ci]:
                                if c == 1:
                                    nc.vector.tensor_copy(
                                        out=classout[:, pos:pos + nseg],
                                        in_=cps[:, loff:loff + nseg])
                                else:
                                    nc.vector.tensor_reduce(
                                        out=classout[:, pos:pos + nseg],
                                        in_=cps[:, loff:loff + nseg * c]
                                            .rearrange("p (n c) -> p n c", c=c),
                                        axis=mybir.AxisListType.X,
                                        op=mybir.AluOpType.add)

                        if layer == 2:
                            for q in range(4):
                                nc.sync.wait_ge(sems[q], 16 * nq[q])
                            nc.sync.dma_start(out=co2_d[:], in_=classout[:])
                            continue
                        if debug_co1:
                            nc.sync.dma_start(out=co1_d[:], in_=classout[:])

                        # ---- un-permute (ap_gather) + tail ----
                        # drain all sprays before the ap_gather library
                        # reload (an IRAM DMA) can fire
                        for q in range(4):
                            nc.gpsimd.wait_ge(sems[q], 16 * nq[q])
                        with tc.tile_pool(name=f"al{layer}_{_rep}", bufs=1) as alp:
                            aligned = alp.tile([128, S], f32)
                            uoff = 0
                            while uoff < S:
                                uw = min(3136, S - uoff)
                                nc.gpsimd.ap_gather(
                                    out_ap=aligned[:, uoff:uoff + uw]
                                        .rearrange("p (n d) -> p n d", d=1),
                                    in_ap=classout[:]
                                        .rearrange("p (n d) -> p n d", d=1),
                                    idxs_ap=unp_t[:, uoff // 16:(uoff + uw) // 16],
                                    channels=128, num_elems=T, d=1, num_idxs=uw)
                                uoff += uw

                            for q in range(4):
                                nc.sync.wait_ge(sems[q], 16 * nq[q])
                            with tc.tile_pool(name=f"po{_rep}", bufs=3) as po, \
                                 tc.tile_pool(name=f"pp{_rep}", bufs=2,
                                              space="PSUM") as pp:
                                for off, w in widths():
                                    agg = pp.tile([F, TILE], f32, tag="agg")
                                    nc.tensor.matmul(
                                        out=agg[:, :w], lhsT=pcomb_t[:],
                                        rhs=aligned[:, off:off + w],
                                        start=True, stop=True)
                                    own = po.tile([F, TILE], bf16, tag="own")
                                    nc.sync.dma_start(out=own[:, :w],
                                                      in_=cc_in1[:, off:off + w])
                                    dvr = po.tile([F, TILE], f32, tag="dvr")
                                    nc.sync.dma_start(
                                        out=dvr[:, :w],
                                        in_=dinv_in_d[0:1, off:off + w]
                                            .to_broadcast([F, w]))
                                    t0 = po.tile([F, TILE], f32, tag="t0")
                                    nc.vector.tensor_add(out=t0[:, :w],
                                                         in0=agg[:, :w],
                                                         in1=own[:, :w])
                                    nc.vector.tensor_tensor(
                                        out=t0[:, :w], in0=t0[:, :w],
                                        in1=dvr[:, :w],
                                        op=mybir.AluOpType.mult)
                                    h1 = po.tile([F, TILE], f32, tag="h1")
                                    nc.scalar.activation(out=h1[:, :w],
                                                         in_=t0[:, :w],
                                                         func=AF.Relu,
                                                         bias=b1_t[:, 0:1])
                                    t2 = pp.tile([C8, TILE], f32, tag="t2")
                                    nc.tensor.matmul(out=t2[:, :w], lhsT=w2_t[:],
                                                     rhs=h1[:, :w],
                                                     start=True, stop=True)
                                    g2 = po.tile([C8, TILE], f32, tag="g2")
                                    nc.vector.tensor_tensor(
                                        out=g2[:, :w], in0=t2[:, :w],
                                        in1=dvr[:C8, :w],
                                        op=mybir.AluOpType.mult)
                                    g2b = po.tile([F, TILE], bf16, tag="g2b")
                                    nc.vector.memset(g2b[:, :w], 0.0)
                                    nc.vector.tensor_copy(out=g2b[:C8, :w],
                                                          in_=g2[:, :w])
                                    nc.sync.dma_start(
                                        out=cc_in2[:, off:off + w],
                                        in_=g2b[:, :w])
                                    nc.sync.dma_start(
                                        out=g2_d[:, off:off + w],
                                        in_=g2[:, :w])
    nc.compile()
    return nc


def _make_selm():
    import ml_dtypes
    selm = np.zeros((128, 1024), np.float32)
    for g in range(8):
        for p in range(128):
            if p // 16 == g:
                selm[p, 128 * g + p] = 1.0
    return selm.astype(ml_dtypes.bfloat16)


def make_const_inputs(W1, b1, W2, b2):
    import ml_dtypes
    pcomb = np.zeros((128, F), np.float32)
    for g in range(NG):
        for f in range(F):
            pcomb[16 * g + f, f] = 1.0
    w2p = np.zeros((F, C8), np.float32); w2p[:, :7] = np.asarray(W2, np.float32)
    return {
        "W1": np.asarray(W1, np.float32).astype(ml_dtypes.bfloat16),
        "b1": np.asarray(b1, np.float32).reshape(F, 1),
        "W2": w2p,
        "pcomb": pcomb,
        "identb": np.eye(128, dtype=np.float32).astype(ml_dtypes.bfloat16),
        "selm": _make_selm(),
    }


def prepare_all(x, edge_index, W1, b1, W2, b2):
    import ml_dtypes
    per_core, meta = preprocess(edge_index)
    consts = make_const_inputs(W1, b1, W2, b2)
    xt = np.zeros((X, V), np.float32)
    xt[:, :N_REAL] = np.asarray(x, np.float32).T
    in_maps = []
    for k in range(NC):
        m = dict(per_core[k])
        m.update(consts)
        # fold the source-side deg^-1/2 into xt: g1 = (dinv*x)@W1
        xs = xt[:, k * S:(k + 1) * S] * m["dinv_in"][0][None, :]
        m["xt"] = np.ascontiguousarray(xs.astype(ml_dtypes.bfloat16))
        in_maps.append(m)
    return in_maps, meta


# ---------------------------------------------------------------- runner
class SpmdRunner:
    def __init__(self, nc, n_cores=NC):
        import jax
        import concourse.mybir as mybir
        from concourse import bass2jax
        from jax.sharding import Mesh, PartitionSpec
        from jax.experimental.shard_map import shard_map
        bass2jax.install_neuronx_cc_hook()
        self.jax = jax
        self.n_cores = n_cores
        partition_name = nc.partition_id_tensor.name if nc.partition_id_tensor else None
        in_names, out_names, out_avals, zero_outs = [], [], [], []
        for alloc in nc.m.functions[0].allocations:
            if not isinstance(alloc, mybir.MemoryLocationSet):
                continue
            name = alloc.memorylocations[0].name
            if alloc.kind == "ExternalInput":
                if name != partition_name:
                    in_names.append(name)
            elif alloc.kind == "ExternalOutput":
                shape = tuple(alloc.tensor_shape)
                dtype = mybir.dt.np(alloc.dtype)
                out_names.append(name)
                out_avals.append(jax.core.ShapedArray(shape, dtype))
                zero_outs.append(np.zeros(shape, dtype))
        self.in_names, self.out_names = in_names, out_names
        self.out_avals, self.zero_outs = out_avals, zero_outs
        n_params, n_outs = len(in_names), len(out_names)
        all_in_names = list(in_names) + list(out_names)
        if partition_name is not None:
            all_in_names.append(partition_name)

        def _body(*args):
            operands = list(args)
            if partition_name is not None:
                operands.append(bass2jax.partition_id_tensor())
            outs = bass2jax._bass_exec_p.bind(
                *operands,
                out_avals=tuple(out_avals),
                in_names=tuple(all_in_names),
                out_names=tuple(out_names),
                lowering_input_output_aliases=(),
                sim_require_finite=False,
                sim_require_nnan=False,
                nc=nc,
            )
            return tuple(outs)

        devices = jax.devices()[:n_cores]
        self.mesh = Mesh(np.asarray(devices), ("core",))
        in_specs = (PartitionSpec("core"),) * (n_params + n_outs)
        out_specs = (PartitionSpec("core"),) * n_outs
        self.fn = jax.jit(
            shard_map(_body, mesh=self.mesh, in_specs=in_specs,
                      out_specs=out_specs, check_rep=False),
            keep_unused=True,
        )

    def _concat(self, in_maps):
        n = self.n_cores
        per_core = [[np.asarray(m[name]) for name in self.in_names] for m in in_maps]
        concat_in = [np.concatenate([per_core[c][i] for c in range(n)], axis=0)
                     for i in range(len(self.in_names))]
        concat_zeros = [np.zeros((n * z.shape[0], *z.shape[1:]), z.dtype)
                        for z in self.zero_outs]
        return concat_in + concat_zeros

    def __call__(self, in_maps):
        jax = self.jax
        out = self.fn(*self._concat(in_maps))
        jax.block_until_ready(out)
        n = self.n_cores
        return [
            {name: np.asarray(out[i]).reshape(n, *self.out_avals[i].shape)[c]
             for i, name in enumerate(self.out_names)}
            for c in range(n)
        ]

    def time_it(self, in_maps, reps=5):
        import time
        jax = self.jax
        from jax.sharding import NamedSharding, PartitionSpec
        sh = NamedSharding(self.mesh, PartitionSpec("core"))
        args = [jax.device_put(a, sh) for a in self._concat(in_maps)]
        out = self.fn(*args); jax.block_until_ready(out)
        ts = []
        for _ in range(reps):
            t0 = time.perf_counter()
            out = self.fn(*args)
            jax.block_until_ready(out)
            ts.append(time.perf_counter() - t0)
        return min(ts), ts


def postprocess(results, meta, in_maps, b2):
    """Finish layer 2 on host: un-permute + combine the 8 per-group partial
    sums from classout2, add the self-loop term, scale, bias, log_softmax."""
    pos = meta["pos"]
    lg_parts = []
    for k in range(NC):
        co2 = np.asarray(results[k]["co2_t"])      # [128, T]
        g2 = np.asarray(results[k]["g2_t"])        # [C8, S]
        dinv = np.asarray(in_maps[k]["dinv_in"])[0]  # [S]
        agg = np.zeros((S, C8), np.float32)
        for g in range(NG):
            agg += co2[16 * g:16 * g + C8, pos[k][g]].T
        lg_parts.append((agg + g2.T) * dinv[:, None])
    lg = np.concatenate(lg_parts, axis=0)[:N_REAL, :7] + b2[None, :7]
    m = lg.max(axis=1, keepdims=True)
    ls = lg - (m + np.log(np.exp(lg - m).sum(axis=1, keepdims=True)))
    return lg.astype(np.float32), ls.astype(np.float32)


_CACHE = {}


def kernel(x, edge_index, W1, b1, W2, b2):
    in_maps, meta = prepare_all(x, edge_index, W1, b1, W2, b2)
    key = (meta["L"], meta["T"],
           tuple(tuple(tuple(r) for r in ops) for ops in meta["plan"]))
    if key not in _CACHE:
        nc = build_kernel(meta)
        _CACHE[key] = SpmdRunner(nc)
    runner = _CACHE[key]
    results = runner(in_maps)
    lg, ls = postprocess(results, meta, in_maps, np.asarray(b2, np.float32))
    return lg, ls
